# revision 14
# baseline (speedup 1.0000x reference)
"""AttentivePooling Trainium2 kernel (v2: fp16 streaming).

Reference semantics (h_all: [T, B, D] f32, xin unused):
    h_last = h_all[-1]                       # [B, D]
    a[b, t] = <h_all[t, b, :], h_last[b, :]> / sqrt(D)
    r = relu(a)
    w = r / (sum_t r + 1e-9)
    out[b, d] = sum_t w[b, t] * h_all[t, b, d]

Strategy: data-parallel over B across 8 cores (8 batches/core, no
collectives).  Per core, HBM traffic is 32 MB and is the roofline, so
everything else is arranged to stay off the DMA critical path:

  - h is streamed in 16 T-slices of 2 MB.  Each SWDGE DMA casts
    f32 -> fp16 inline (128 descriptors x 16 KB, full rate) into a
    [128(t), 8(b), 512(d)] tile; 4 slice buffers give 3 slices of
    prefetch.  fp16 halves every downstream engine's cost while the
    2e-2 error budget dwarfs fp16's ~1e-3.
  - h_last broadcast: tiny DMA of h[T-1] -> [1, 8*512] fp16, then PE
    rank-1 matmuls (scale_row x hl row) broadcast it across the 128
    partitions with 1/sqrt(D) folded in; ACT/DVE copy PSUM -> SBUF
    fp16.  No 128x re-read of HBM for broadcasts.
  - scores per slice: DVE fp16 multiplies (2x packed mode), reduction
    over d split ACT activation-accum / DVE tensor_reduce.  relu via
    one DVE tensor_scalar_max into fp16 weight columns.
  - pooling: PE accumulates sum_t w_t * h_t incrementally across all
    16 slices; batch b accumulates in PSUM bank b//2 at partition
    offset 64*(b%2) (engine APs may only start at partition 0/32/64),
    8 independent accumulation groups, so no second pass over h.
  - epilogue: weight sums land at partitions 0/64 directly by using
    [128, 65] lhsT tiles whose cols 0/64 hold the per-partition weight
    sums; add eps, reciprocal, one per-partition tensor_scalar
    multiply per bank pair, strided 2-row stores.
"""

import numpy as np
from contextlib import ExitStack

import concourse.bass as bass
import concourse.tile as tile
from concourse import bacc, mybir
from concourse.bass_utils import run_bass_kernel_spmd

T, B, D = 2048, 64, 512
NCORES = 8
BPC = B // NCORES  # batches per core
P = 128
S = 16  # T-slices per core; slice = [P, BPC, D]
PRE = 5  # slices of DMA prefetch (slice pool bufs = PRE + 1)
SCALE = float(1.0 / np.sqrt(np.float32(D)))
ACT_REDUCE = 5  # batches 0..4 reduce on ACT, rest on DVE
GP_MULT = 0  # GPSIMD does no compute: it shares an exclusive SBUF port pair with DVE perf-mode ops and SWDGE descriptor gen
NACC = BPC // 2  # PSUM accumulator banks (2 batches per bank, rows 0/64)

_nc_cache = None


def _build():
    global _nc_cache
    if _nc_cache is not None:
        return _nc_cache
    nc = bacc.Bacc("TRN2", debug=False, target_bir_lowering=False, num_devices=NCORES)
    h = nc.dram_tensor("h", [T, BPC, D], mybir.dt.float32, kind="ExternalInput")
    out = nc.dram_tensor("out", [BPC, D], mybir.dt.float32, kind="ExternalOutput")
    h_ap = h.ap()
    out_ap = out.ap()
    f32 = mybir.dt.float32
    f16 = mybir.dt.float16

    with tile.TileContext(nc) as tc:
        with ExitStack() as ctx:
            hpool = ctx.enter_context(tc.tile_pool(name="h", bufs=PRE + 1))
            constp = ctx.enter_context(tc.tile_pool(name="const", bufs=1))
            smallp = ctx.enter_context(tc.tile_pool(name="small", bufs=2))
            tmpp = ctx.enter_context(tc.tile_pool(name="tmp", bufs=8))
            psbcp = ctx.enter_context(tc.tile_pool(name="psb", bufs=2, space="PSUM"))
            psaccp = ctx.enter_context(tc.tile_pool(name="psa", bufs=1, space="PSUM"))
            pszp = ctx.enter_context(tc.tile_pool(name="psz", bufs=1, space="PSUM"))

            # constants
            scale_row = constp.tile([1, P], f16)
            nc.vector.memset(scale_row[:], SCALE)
            ones_col = constp.tile([P, 1], f32)
            nc.vector.memset(ones_col[:], 1.0)
            eps_tile = constp.tile([P, 1], f32)
            nc.vector.memset(eps_tile[:], 1e-9)

            # persistent SBUF state
            hlb_all = constp.tile([P, BPC, D], f16)  # broadcast h_last per batch
            scr_all = constp.tile([P, S, BPC], f32)  # raw scores
            w_all = constp.tile([P, S, BPC], f16)  # relu'd scores (weights)
            # weight-sum carriers: cols 0/64 of tile t hold Z cols for
            # batches 2t / 2t+1, so the PE drops Z at partitions 0/64
            zw = []
            for t in range(NACC):
                zw_t = constp.tile([P, 65], f32, name=f"zw{t}")
                nc.vector.memset(zw_t[:], 0.0)
                zw.append(zw_t)

            # h_last [1, BPC*D] fp16 (flat on partition 0)
            hl_flat = constp.tile([1, BPC * D], f16)
            nc.gpsimd.dma_start(hl_flat[:], h_ap[T - 1, :, :])

            def load_slice(s):
                t = hpool.tile([P, BPC, D], f16, tag="hsl", name="h_sl")
                nc.gpsimd.dma_start(t[:], h_ap[s * P : (s + 1) * P, :, :])
                return t

            h_tiles = {}
            for s in range(PRE):
                h_tiles[s] = load_slice(s)

            # broadcast h_last[b] across partitions via PE rank-1 matmul,
            # with the 1/sqrt(D) score scale folded into the ones vector
            for b in range(BPC):
                psb = psbcp.tile([P, D], f32, tag="psb")
                nc.tensor.matmul(
                    psb[:],
                    scale_row[:],
                    hl_flat[0:1, b * D : (b + 1) * D],
                    start=True,
                    stop=True,
                )
                if b % 2 == 0:
                    nc.scalar.copy(hlb_all[:, b, :], psb[:])
                else:
                    nc.vector.tensor_scalar_mul(hlb_all[:, b, :], psb[:], 1.0)

            # pooled accumulators: batch b -> acc[b//2] partition 64*(b%2)
            acc = [psaccp.tile([P, D], f32, name=f"acc{t}") for t in range(NACC)]

            for s in range(S):
                if s + PRE < S:
                    h_tiles[s + PRE] = load_slice(s + PRE)
                h_sl = h_tiles.pop(s)

                if s == S - 1:
                    # partial Z over weight cols 0..S-2 (col S-1 added below)
                    for b in range(BPC):
                        col = 64 * (b % 2)
                        nc.vector.tensor_reduce(
                            zw[b // 2][:, col : col + 1],
                            w_all[:, 0 : S - 1, b],
                            mybir.AxisListType.X,
                            mybir.AluOpType.add,
                        )

                for b in range(BPC):
                    tmp = tmpp.tile([P, D], f16, tag="tmp")
                    meng = nc.gpsimd if b < GP_MULT else nc.vector
                    meng.tensor_tensor(
                        tmp[:],
                        h_sl[:, b, :],
                        hlb_all[:, b, :],
                        mybir.AluOpType.mult,
                    )
                    if b < ACT_REDUCE:
                        nc.scalar.activation(
                            tmp[:],
                            tmp[:],
                            mybir.ActivationFunctionType.Copy,
                            accum_out=scr_all[:, s, b : b + 1],
                        )
                    else:
                        nc.vector.tensor_reduce(
                            scr_all[:, s, b : b + 1],
                            tmp[:],
                            mybir.AxisListType.X,
                            mybir.AluOpType.add,
                        )

                nc.vector.tensor_scalar_max(w_all[:, s, :], scr_all[:, s, :], 0.0)

                for b in range(BPC):
                    row = 64 * (b % 2)
                    nc.tensor.matmul(
                        acc[b // 2][row : row + 1, :],
                        w_all[:, s, b : b + 1],
                        h_sl[:, b, :],
                        start=(s == 0),
                        stop=(s == S - 1),
                    )

            # epilogue: fold weight col S-1 into Z (partials done in-loop)
            for b in range(BPC):
                col = 64 * (b % 2)
                nc.vector.tensor_tensor(
                    zw[b // 2][:, col : col + 1],
                    zw[b // 2][:, col : col + 1],
                    w_all[:, S - 1, b : b + 1],
                    mybir.AluOpType.add,
                )
            pz = pszp.tile([65, NACC], f32)
            for t in range(NACC):
                nc.tensor.matmul(
                    pz[:, t : t + 1], zw[t][:], ones_col[:], start=True, stop=True
                )
            zeps = smallp.tile([P, NACC], f32, tag="zeps")
            nc.scalar.activation(
                zeps[0:65, :],
                pz[:],
                mybir.ActivationFunctionType.Identity,
                bias=eps_tile[0:65, 0:1],
            )
            zrec = smallp.tile([P, NACC], f32, tag="zrec")
            nc.vector.reciprocal(zrec[0:65, :], zeps[0:65, :])
            for t in range(NACC):
                res = smallp.tile([P, D], f32, tag=f"res{t}")
                if t % 2 == 0:
                    nc.vector.tensor_scalar_mul(res[:], acc[t][:], zrec[:, t : t + 1])
                else:
                    nc.scalar.activation(
                        res[:],
                        acc[t][:],
                        mybir.ActivationFunctionType.Copy,
                        scale=zrec[:, t : t + 1],
                    )
                nc.sync.dma_start(out_ap[2 * t : 2 * t + 2, :], res[0:128:64, :])

    nc.finalize()
    _nc_cache = nc
    return nc


def _run(h_all: np.ndarray, trace: bool = False):
    nc = _build()
    h_all = np.ascontiguousarray(np.asarray(h_all), dtype=np.float32)
    assert h_all.shape == (T, B, D)
    in_maps = [
        {"h": np.ascontiguousarray(h_all[:, c * BPC : (c + 1) * BPC, :])}
        for c in range(NCORES)
    ]
    r = run_bass_kernel_spmd(nc, in_maps, list(range(NCORES)), trace=trace)
    out = np.concatenate([r.results[c]["out"] for c in range(NCORES)], axis=0)
    return out, r


def kernel(h_all: np.ndarray, xin: np.ndarray | None = None) -> np.ndarray:
    out, _ = _run(h_all)
    return out
